# revision 35
# baseline (speedup 1.0000x reference)
"""GCNBlock (2-graph GCN, depth 4) on 8 Trainium2 NeuronCores — v2.3.

Architecture (v2 lineage — measured SDMA-descriptor-bound):
  - Aggregation-first: per dst block, segment-sum of gathered fp16 node rows
    via one-hot sel matmuls into PSUM; the [F,F] weights applied after.
  - One fp16 node table y = dinv1*x per layer, split A/B halves, AllGathered
    per layer (A fires mid-block-loop to overlap; B at layer end).
  - Main+ctrl graphs share one gather stream per (block, half): mixed chunks
    are consumed by two sel matmuls from one gathered tile. Main sel 0/1
    built on DVE (batched is_equal vs dst-local columns); ctrl sels
    (value = dinv2[src]/dinv1[src]) are host-built and streamed fp16.
  - Gather calls pack 8 chunks (1024 idxs = SWDGE ring cap), round-robin
    over 4 SWDGE queues. ~36ns/descriptor (HBM 256B random) is the wall.
v2.3 deltas vs v2 (1578us baseline):
  - mC PSUM->SBUF copy moved DVE -> Scalar engine.
  - diag (self-loop) slabs host-precomputed, one DMA (kills 98 DVE
    tensor_scalar @ ~2.7us fixed cost each).
  - next-layer shard rows written straight into an SBUF ping-pong buffer
    (xsh for diags) and flushed to DRAM once per half for the AllGather:
    removes 147 per-block shard-write DMAs + 8 strided xsh reloads.
"""

import math
import os

import numpy as np

F = 128
DEPTH = 4
P = 128
# table pieces (block ranges): staggered AllGathers; piece row counts must
# satisfy 8*rows <= 32768 (int16 gather idx)
PIECES = [(0, 25), (25, 49)]
NH = len(PIECES)
CALL_CAP = 8           # chunks per dma_gather call (1024 idxs = ring cap)

LAST_INFO = {}


class _Cfg:
    def __init__(self, n_nodes, n_cores=8):
        assert n_nodes % n_cores == 0
        self.N = n_nodes
        self.NCORES = n_cores
        self.SH = n_nodes // n_cores
        self.NBLK = math.ceil(self.SH / P)
        self.SHPAD = self.NBLK * P
        self.NQ = int(os.environ.get("GCN_NQUEUES", "4"))


def _prep_graph(edge_index, cfg):
    src = np.asarray(edge_index[0]).astype(np.int64)
    dst = np.asarray(edge_index[1]).astype(np.int64)
    deg = (np.bincount(dst, minlength=cfg.N) + 1).astype(np.float64)
    dinv = (1.0 / np.sqrt(deg)).astype(np.float32)

    core = dst // cfg.SH
    loc = dst - core * cfg.SH
    blk = loc // P
    dl = (loc % P).astype(np.int32)
    score = src // cfg.SH
    sloc = src - score * cfg.SH              # local row in source shard
    sblk = sloc // P
    half = np.zeros(len(src), np.int64)
    idxv = np.zeros(len(src), np.int64)
    for pi, (lo, hi) in enumerate(PIECES):
        m = (sblk >= lo) & (sblk < hi)
        half[m] = pi
        idxv[m] = score[m] * (hi - lo) * P + sloc[m] - lo * P
    idxv = idxv.astype(np.int32)
    assert idxv.max() < 32768
    return core, blk, half, idxv, dl, dinv, src


def _build_streams(x, edge_index, control_edge_index, cfg):
    """Host prep producing unified SPMD streams (cross-block call packing)."""
    graphs = [_prep_graph(edge_index, cfg),
              _prep_graph(control_edge_index, cfg)]
    dinvs = [graphs[0][5], graphs[1][5]]
    ratio = dinvs[1] / dinvs[0]   # dinv2/dinv1 per node

    buckets = {}
    for g in range(2):
        core, blk, half, idxv, dl, _, srcg = graphs[g]
        if g == 0:
            norm = np.ones(len(dl), np.float32)    # main sel value: 1.0
        else:
            norm = ratio[srcg]                     # ctrl: ratio[src]
        key = ((core * cfg.NBLK + blk) * NH + half)
        order = np.argsort(key, kind="stable")
        ks = key[order]
        bounds = np.searchsorted(
            ks, np.arange(cfg.NCORES * cfg.NBLK * NH + 1))
        for r in range(cfg.NCORES):
            for b in range(cfg.NBLK):
                for h in range(NH):
                    gi = (r * cfg.NBLK + b) * NH + h
                    sidx = order[bounds[gi]:bounds[gi + 1]]
                    buckets[(r, b, h, g)] = (idxv[sidx], dl[sidx], norm[sidx])

    # unified chunk structure per (b, h): nch = max over cores; per-chunk
    # graph-set = union over cores
    chmeta = {}            # (b, h) -> list of gsets
    for b in range(cfg.NBLK):
        for h in range(NH):
            nchs, bnds = [], []
            for r in range(cfg.NCORES):
                ne0 = len(buckets[(r, b, h, 0)][0])
                ne1 = len(buckets[(r, b, h, 1)][0])
                nchs.append(max(1, math.ceil((ne0 + ne1) / P)))
                bnds.append((ne0, ne0 + ne1))
            nch = max(nchs)
            gsets = []
            for c in range(nch):
                s0, s1 = c * P, (c + 1) * P
                gs = set()
                for r in range(cfg.NCORES):
                    ne0, ne = bnds[r]
                    if min(s1, ne0) > s0:
                        gs.add(0)
                    if min(s1, ne) > max(s0, ne0):
                        gs.add(1)
                if not gs:
                    gs = {0}
                gsets.append(sorted(gs))
            chmeta[(b, h)] = gsets

    # per-half chunk streams (block-ordered) -> calls of <= CALL_CAP chunks
    calls = [[] for _ in range(NH)]  # per h: (k, start_block, [(b, gset)])
    blockrefs = [[] for _ in range(cfg.NBLK)]   # per b: (h, ci, cc, gset)
    slot_of = {}           # (h, ci, cc) -> global slot index
    nslot = 0
    for h in range(NH):
        stream = [(b, gs) for b in range(cfg.NBLK)
                  for gs in chmeta[(b, h)]]
        c = 0
        while c < len(stream):
            k = min(CALL_CAP, len(stream) - c)
            ci = len(calls[h])
            chunks = stream[c:c + k]
            calls[h].append((k, chunks[0][0], chunks))
            for cc, (b, gs) in enumerate(chunks):
                blockrefs[b].append((h, ci, cc, gs))
                slot_of[(h, ci, cc)] = nslot
                nslot += 1
            c += k
    for b in range(cfg.NBLK):
        blockrefs[b].sort(key=lambda t: (t[0], t[1], t[2]))

    # ctrl sel stream grouped per block in blockref order; main sels are
    # built on-device (DVE is_equal vs dlm) so only dst-locals ship.
    nsel16 = sum(1 for b in range(cfg.NBLK) for (_h, _ci, _cc, gs)
                 in blockrefs[b] if 1 in gs)

    idx16s, dlms, sel16s = [], [], []
    for r in range(cfg.NCORES):
        idx_slots = np.zeros(nslot * P, np.int16)
        dlm_slots = np.full((P, nslot), 255.0, np.float16)
        selsB = np.zeros((nsel16, P, P), np.float16)
        chunk_pos = {}     # (b, h) -> next chunk ordinal within the group
        for h in range(NH):
            for ci, (k, _sb, chunks) in enumerate(calls[h]):
                for cc, (b, _gs) in enumerate(chunks):
                    j = chunk_pos.get((b, h), 0)
                    chunk_pos[(b, h)] = j + 1
                    i0, d0, _n0 = buckets[(r, b, h, 0)]
                    i1, _d1, _n1 = buckets[(r, b, h, 1)]
                    idxs = np.concatenate([i0, i1]).astype(np.int16)
                    lo, hi = j * P, min((j + 1) * P, len(idxs))
                    base = slot_of[(h, ci, cc)] * P
                    if hi > lo:
                        idx_slots[base:base + hi - lo] = idxs[lo:hi]
                    mhi = min(hi, len(i0))
                    if mhi > lo:
                        sl = slot_of[(h, ci, cc)]
                        dlm_slots[0:mhi - lo, sl] = d0[lo:mhi]
        # fill ctrl sels in blockref order
        si16 = 0
        for b in range(cfg.NBLK):
            seen = {}
            for (h, ci, cc, gs) in blockrefs[b]:
                j = seen.get(h, 0)
                seen[h] = j + 1
                i0, d0, n0 = buckets[(r, b, h, 0)]
                i1, d1, n1 = buckets[(r, b, h, 1)]
                dls = np.concatenate([d0, d1])
                nrms = np.concatenate([n0, n1])
                ne0, ne = len(i0), len(i0) + len(i1)
                s0, s1 = j * P, (j + 1) * P
                if 1 in gs:
                    lo = max(s0, ne0)
                    hi = min(s1, ne)
                    if hi > lo:
                        rows = np.arange(lo, hi) - s0
                        selsB[si16][rows, dls[lo:hi]] = nrms[lo:hi]
                    si16 += 1
        assert si16 == nsel16
        wrapped = idx_slots.reshape(-1, 16).T.copy()
        idx16s.append(np.tile(wrapped, (8, 1)))
        dlms.append(dlm_slots)
        sel16s.append(np.ascontiguousarray(
            selsB.transpose(1, 0, 2).reshape(P, nsel16 * P)))

    # layer-0 tables (one per piece, core-concatenated) + resident shard
    prow = [(hi - lo) * P for lo, hi in PIECES]
    xs = (np.asarray(x, np.float32) * dinvs[0][:, None]).astype(np.float32)
    xr = xs.reshape(cfg.NCORES, cfg.SH, F)
    xpads = [np.zeros((cfg.NCORES * prow[pi], F), np.float16)
             for pi in range(NH)]
    xsh0s = []
    for r in range(cfg.NCORES):
        full = np.zeros((cfg.SHPAD, F), np.float16)
        full[:cfg.SH] = xr[r].astype(np.float16)
        for pi, (lo, hi) in enumerate(PIECES):
            xpads[pi][r * prow[pi]:(r + 1) * prow[pi]] = \
                full[lo * P:hi * P]
        xsh0s.append(full)

    # host-precomputed diag slabs [P, NBLK*2, P]: diag per (block, graph):
    #   g=0: 1.0 (y row; end-scaled dinv1[d]); g=1: ratio[d]
    diags, dcols = [], []
    eye = np.eye(P, dtype=np.float32)
    for r in range(cfg.NCORES):
        dg = np.zeros((P, cfg.NBLK * 2, P), np.float16)
        dc_ = np.zeros((P, cfg.NBLK * 2), np.float32)
        for b in range(cfg.NBLK):
            i0 = r * cfg.SH + b * P
            n = min(P, cfg.SH - b * P)
            val1 = np.zeros(P, np.float32)
            val2 = np.zeros(P, np.float32)
            val1[:n] = 1.0
            val2[:n] = ratio[i0:i0 + n]
            dg[:, b * 2 + 0, :] = (eye * val1[:, None]).astype(np.float16)
            dg[:, b * 2 + 1, :] = (eye * val2[:, None]).astype(np.float16)
            dc_[:n, b * 2 + 0] = dinvs[0][i0:i0 + n]
            dc_[:n, b * 2 + 1] = dinvs[1][i0:i0 + n]
        diags.append(dg.reshape(P, cfg.NBLK * 2 * P))
        dcols.append(dc_)

    meta = dict(calls=calls, blockrefs=blockrefs, slot_of=slot_of,
                nslot=nslot, nsel16=nsel16)
    return (meta, idx16s, dlms, sel16s, xpads, xsh0s,
            diags, dcols)


def _build_program(cfg, meta, depth):
    import concourse.bacc as bacc
    import concourse.mybir as mybir
    import concourse.tile as tile

    dtH = mybir.dt.float16
    dt32 = mybir.dt.float32
    AT = mybir.AluOpType

    calls = meta["calls"]
    blockrefs = meta["blockrefs"]
    slot_of = meta["slot_of"]
    nslot = meta["nslot"]
    nsel16 = meta["nsel16"]

    nc = bacc.Bacc(
        "TRN2", debug=False, num_devices=cfg.NCORES,
        num_swdge_queues=cfg.NQ,
        dynamic_dma_scratch_size=16384,
    )

    prow = [(hi - lo) * P for lo, hi in PIECES]
    xpad_t = [nc.dram_tensor(f"xpad{pi}", [cfg.NCORES * prow[pi], F], dtH,
                             kind="ExternalInput")
              for pi in range(NH)]
    idx_t = nc.dram_tensor("idx16", [P, nslot * P // 16], mybir.dt.int16,
                           kind="ExternalInput")
    dlm_t = nc.dram_tensor("dlm", [P, nslot], dtH, kind="ExternalInput")
    sel16_t = nc.dram_tensor("sel16", [P, nsel16 * P], dtH,
                             kind="ExternalInput")
    diag_t = nc.dram_tensor("diag", [P, cfg.NBLK * 2 * P], dtH,
                            kind="ExternalInput")
    iotar_t = nc.dram_tensor("iotar", [P, P], dtH, kind="ExternalInput")
    w1_t = nc.dram_tensor("w1", [F, depth * F], dtH, kind="ExternalInput")
    w2_t = nc.dram_tensor("w2", [F, depth * F], dtH, kind="ExternalInput")
    xsh0_t = nc.dram_tensor("xsh0", [cfg.SHPAD, F], dtH, kind="ExternalInput")
    dcol_t = nc.dram_tensor("dcol", [P, cfg.NBLK * 2], dt32,
                            kind="ExternalInput")
    out_t = nc.dram_tensor("out", [cfg.SHPAD, F], dt32, kind="ExternalOutput")

    # per-block ctrl sel slab sizes
    sel16cnt, sel16off = [], []
    o16 = 0
    for b in range(cfg.NBLK):
        n16 = sum(1 for (_h, _ci, _cc, gs) in blockrefs[b] if 1 in gs)
        sel16off.append(o16)
        sel16cnt.append(n16)
        o16 += n16
    NS16MAX = max(sel16cnt)

    with tile.TileContext(nc) as tc:
        with (
            tc.tile_pool(name="const", bufs=1) as cpool,
            tc.tile_pool(name="gather", bufs=20) as gpool,
            tc.tile_pool(name="selb", bufs=12) as s8pool,
            tc.tile_pool(name="sel", bufs=4) as spool,
            tc.tile_pool(name="msb", bufs=4) as mpool,
            tc.tile_pool(name="xn", bufs=3) as xpool,
            tc.tile_pool(name="xsh", bufs=2) as xshpool,
            tc.tile_pool(name="pm", bufs=6, space="PSUM") as pmpool,
            tc.tile_pool(name="po", bufs=2, space="PSUM") as popool,
            tc.tile_pool(name="shard", bufs=6, space="DRAM") as shpool,
            tc.tile_pool(name="table", bufs=6, space="DRAM") as tbpool,
        ):
            idx_sb = cpool.tile([P, nslot * P // 16], mybir.dt.int16)
            nc.sync.dma_start(out=idx_sb[:], in_=idx_t[:])
            iotar_sb = cpool.tile([P, P], dtH)
            nc.sync.dma_start(out=iotar_sb[:], in_=iotar_t[:])
            iocall_sb = cpool.tile([P, CALL_CAP, P], dtH)
            for _c in range(CALL_CAP):
                nc.vector.tensor_copy(out=iocall_sb[:, _c, :],
                                      in_=iotar_sb[:])
            dlm_sb = cpool.tile([P, nslot], dtH)
            nc.sync.dma_start(out=dlm_sb[:], in_=dlm_t[:])
            w1_sb = cpool.tile([F, depth * F], dtH)
            w2_sb = cpool.tile([F, depth * F], dtH)
            nc.sync.dma_start(out=w1_sb[:], in_=w1_t[:])
            nc.sync.dma_start(out=w2_sb[:], in_=w2_t[:])
            dcol_sb = cpool.tile([P, cfg.NBLK * 2], dt32)
            nc.sync.dma_start(out=dcol_sb[:], in_=dcol_t[:])
            diag_sb = cpool.tile([P, cfg.NBLK * 2, P], dtH)
            nc.sync.dma_start(
                out=diag_sb[:],
                in_=diag_t[:].rearrange("p (b q) -> p b q", q=P))

            qrr = [0]
            prev_tab = [None] * NH
            xsh_cur = None
            for l in range(depth):
                if l < depth - 1:
                    sh = [shpool.tile([prow[pi], F], dtH, tag=f"sh{pi}",
                                      name=f"sh{pi}")
                          for pi in range(NH)]
                    # next-layer resident shard, filled block by block
                    xsh_nxt = xshpool.tile([P, cfg.NBLK, F], dtH, tag="xsh",
                                           name="xsh_nxt")
                if l == 0:
                    xsh_cur = xshpool.tile([P, cfg.NBLK, F], dtH, tag="xsh",
                                           name="xsh0")
                    nc.sync.dma_start(
                        out=xsh_cur[:],
                        in_=xsh0_t[:].rearrange("(b p) f -> p b f", p=P))

                next_tab = [None] * NH
                nxt = [0] * NH   # per-piece next-call-to-issue pointer
                gts = {}         # (h, ci) -> gather tile
                sel8s_t = {}     # (h, ci) -> built main-sel tile
                LAS = [5, 2]     # per-piece issue lookahead
                for b in range(cfg.NBLK):
                    sel16sb = spool.tile([P, NS16MAX, P], dtH, tag="sel16",
                                         name="sel16sb")
                    nc.sync.dma_start(
                        out=sel16sb[:, 0:sel16cnt[b], :],
                        in_=sel16_t[:, sel16off[b] * P:
                                    (sel16off[b] + sel16cnt[b]) * P
                                    ].rearrange("p (n q) -> p n q", q=P))

                    # issue gather calls (cross-block packing); early
                    # pieces run ahead (their tables land mid-prev-layer)
                    for h in range(NH):
                        lim = b + LAS[h]
                        while (nxt[h] < len(calls[h])
                               and calls[h][nxt[h]][1] <= lim):
                            ci = nxt[h]
                            k, _sb, _chunks = calls[h][ci]
                            gt = gpool.tile([P, CALL_CAP, F], dtH, tag="gt",
                                            name="gt")
                            L = k * P
                            s0 = slot_of[(h, ci, 0)]
                            if l == 0:
                                src_ap = xpad_t[h][:]
                            else:
                                src_ap = prev_tab[h][:]
                            nc.gpsimd.dma_gather(
                                gt[:, 0:k, :], src_ap,
                                idx_sb[:, s0 * 8:s0 * 8 + L // 16],
                                L, L, F,
                                queue_num=qrr[0] % cfg.NQ,
                            )
                            qrr[0] += 1
                            gts[(h, ci)] = gt
                            nxt[h] += 1

                    # build main 0/1 sels for calls this block consumes (DVE)
                    for (h, ci, _cc, _gs) in blockrefs[b]:
                        if (h, ci) in sel8s_t:
                            continue
                        k = calls[h][ci][0]
                        s0 = slot_of[(h, ci, 0)]
                        s8 = s8pool.tile([P, CALL_CAP, P], dtH,
                                         tag="sel8c", name="s8")
                        nc.vector.tensor_tensor(
                            out=s8[:, 0:k, :],
                            in0=iocall_sb[:, 0:k, :],
                            in1=dlm_sb[:, s0:s0 + k].to_broadcast(
                                [P, k, P]),
                            op=AT.is_equal,
                        )
                        sel8s_t[(h, ci)] = s8

                    # matmul accumulation: main split across 2 psums (A/B
                    # alternating), ctrl in psum C
                    pmA = pmpool.tile([P, P], dt32, tag="pm", name="pmA")
                    pmB = pmpool.tile([P, P], dt32, tag="pm", name="pmB")
                    pmC = pmpool.tile([P, P], dt32, tag="pm", name="pmC")
                    nA, nB, nC = 1, 0, 1
                    alt = 0
                    for (_h, _ci, _cc, gs) in blockrefs[b]:
                        for g in gs:
                            if g == 0:
                                if alt == 0:
                                    nA += 1
                                else:
                                    nB += 1
                                alt ^= 1
                            else:
                                nC += 1
                    use_B = nB > 0
                    dA, dB, dC = 0, 0, 0
                    nc.tensor.matmul(
                        out=pmA[:], lhsT=xsh_cur[:, b, :],
                        rhs=diag_sb[:, b * 2, :],
                        start=True, stop=(nA == 1),
                    )
                    dA = 1
                    nc.tensor.matmul(
                        out=pmC[:], lhsT=xsh_cur[:, b, :],
                        rhs=diag_sb[:, b * 2 + 1, :],
                        start=True, stop=(nC == 1),
                    )
                    dC = 1
                    sli16 = 0
                    alt = 0
                    for (h, ci, cc, gs) in blockrefs[b]:
                        gt = gts[(h, ci)]
                        for g in gs:
                            if g == 0:
                                rhs = sel8s_t[(h, ci)][:, cc, :]
                                if alt == 0:
                                    tgt = pmA
                                    dA += 1
                                    st = (dA == nA)
                                    first = False
                                else:
                                    tgt = pmB
                                    first = (dB == 0)
                                    dB += 1
                                    st = (dB == nB)
                                alt ^= 1
                            else:
                                rhs = sel16sb[:, sli16, :]
                                sli16 += 1
                                tgt = pmC
                                dC += 1
                                st = (dC == nC)
                                first = False
                            nc.tensor.matmul(
                                out=tgt[:],
                                lhsT=gt[:, cc, :],
                                rhs=rhs,
                                start=first,
                                stop=st,
                            )
                    mA = mpool.tile([P, P], dtH, tag="m", name="mA")
                    mC = mpool.tile([P, P], dtH, tag="m", name="mC")
                    nc.scalar.activation(
                        out=mA[:], in_=pmA[:],
                        func=mybir.ActivationFunctionType.Copy)
                    if use_B:
                        mB = mpool.tile([P, P], dtH, tag="m", name="mB")
                        nc.scalar.activation(
                            out=mB[:], in_=pmB[:],
                            func=mybir.ActivationFunctionType.Copy)
                    nc.scalar.activation(
                        out=mC[:], in_=pmC[:],
                        func=mybir.ActivationFunctionType.Copy)
                    poA = popool.tile([P, P], dt32, tag="po", name="poA")
                    poB = popool.tile([P, P], dt32, tag="po", name="poB")
                    nc.tensor.matmul(out=poA[:], lhsT=mA[:],
                                     rhs=w1_sb[:, l * F:(l + 1) * F],
                                     start=True, stop=not use_B)
                    if use_B:
                        nc.tensor.matmul(out=poA[:], lhsT=mB[:],
                                         rhs=w1_sb[:, l * F:(l + 1) * F],
                                         start=False, stop=True)
                    nc.tensor.matmul(out=poB[:], lhsT=mC[:],
                                     rhs=w2_sb[:, l * F:(l + 1) * F],
                                     start=True, stop=True)
                    # pB = poB * dinv2[dst] (PSUM->SBUF); pt = poA*dinv1 + pB
                    pB = xpool.tile([P, P], dt32, tag="pB", name="pB")
                    nc.scalar.activation(
                        out=pB[:], in_=poB[:],
                        func=mybir.ActivationFunctionType.Copy,
                        scale=dcol_sb[:, b * 2 + 1:b * 2 + 2])
                    pt = xpool.tile([P, P], dt32, tag="pt", name="pt")
                    nc.vector.scalar_tensor_tensor(
                        out=pt[:], in0=poA[:],
                        scalar=dcol_sb[:, b * 2:b * 2 + 1],
                        in1=pB[:], op0=AT.mult, op1=AT.add)
                    if l < depth - 1:
                        # next-layer shard row y1 = relu(dinv1[d] * pt),
                        # written straight into the SBUF ping-pong shard
                        nc.scalar.activation(
                            out=xsh_nxt[:, b, :], in_=pt[:],
                            func=mybir.ActivationFunctionType.Relu,
                            scale=dcol_sb[:, b * 2:b * 2 + 1])
                        for pi, (plo, phi) in enumerate(PIECES):
                            if b != phi - 1:
                                continue
                            # piece complete: flush once + AllGather,
                            # overlapped with remaining blocks' compute
                            nc.sync.dma_start(
                                out=sh[pi][:].rearrange(
                                    "(b p) f -> p b f", p=P),
                                in_=xsh_nxt[:, plo:phi, :])
                            tab = tbpool.tile(
                                [cfg.NCORES * prow[pi], F], dtH,
                                tag=f"tab{pi}", name=f"tab{pi}",
                                addr_space="Shared")
                            nc.gpsimd.collective_compute(
                                "AllGather",
                                mybir.AluOpType.bypass,
                                replica_groups=[list(range(cfg.NCORES))],
                                ins=[sh[pi].opt()],
                                outs=[tab.opt()],
                            )
                            next_tab[pi] = tab
                    else:
                        nc.sync.dma_start(
                            out=out_t[b * P:(b + 1) * P, :], in_=pt[:])

                if l < depth - 1:
                    prev_tab = next_tab
                    xsh_cur = xsh_nxt

    nc.compile()
    return nc


def _run(x, edge_index, control_edge_index, conv_w, conv_b, ctrl_w, ctrl_b,
         cfg, trace=False):
    from concourse.bass_utils import run_bass_kernel_spmd

    depth = int(np.asarray(conv_w).shape[0])
    (meta, idx16s, dlms, sel16s, xpads,
     xsh0s, diags, dcols) = _build_streams(
        x, edge_index, control_edge_index, cfg)
    bsum = (np.asarray(conv_b, np.float32)
            + np.asarray(ctrl_b, np.float32))
    assert not np.any(bsum), "bias path not supported"
    nc = _build_program(cfg, meta, depth)

    w1 = (np.asarray(conv_w, np.float32).transpose(1, 0, 2)
          .reshape(F, depth * F).astype(np.float16))
    w2 = (np.asarray(ctrl_w, np.float32).transpose(1, 0, 2)
          .reshape(F, depth * F).astype(np.float16))
    iotar = np.tile(np.arange(P, dtype=np.float16), (P, 1))

    in_maps = []
    for r in range(cfg.NCORES):
        m = {"idx16": idx16s[r],
             "dlm": dlms[r], "sel16": sel16s[r], "diag": diags[r],
             "w1": w1, "w2": w2, "iotar": iotar,
             "xsh0": xsh0s[r], "dcol": dcols[r]}
        for pi in range(NH):
            m[f"xpad{pi}"] = xpads[pi]
        in_maps.append(m)

    try:
        res = run_bass_kernel_spmd(nc, in_maps, list(range(cfg.NCORES)),
                                   trace=trace)
    except Exception:
        if not trace:
            raise
        res = run_bass_kernel_spmd(nc, in_maps, list(range(cfg.NCORES)),
                                   trace=False)
    LAST_INFO.clear()
    LAST_INFO["exec_time_ns"] = res.exec_time_ns
    LAST_INFO["mean_exec_time_ns"] = res.mean_exec_time_ns
    LAST_INFO["profile_json"] = res.profile_json

    out = np.empty((cfg.N, F), np.float32)
    for r in range(cfg.NCORES):
        out[r * cfg.SH:(r + 1) * cfg.SH] = res.results[r]["out"][:cfg.SH]
    return out


def kernel(x, edge_index, control_edge_index, conv_w, conv_b, ctrl_w, ctrl_b):
    cfg = _Cfg(int(np.asarray(x).shape[0]))
    trace = os.environ.get("GCN_TRACE", "0") == "1"
    return _run(x, edge_index, control_edge_index, conv_w, conv_b,
                ctrl_w, ctrl_b, cfg, trace=trace)


# revision 36
# speedup vs baseline: 1.0106x; 1.0106x over previous
"""GCNBlock (2-graph GCN, depth 4) on 8 Trainium2 NeuronCores — v2.3.

Architecture (v2 lineage — measured SDMA-descriptor-bound):
  - Aggregation-first: per dst block, segment-sum of gathered fp16 node rows
    via one-hot sel matmuls into PSUM; the [F,F] weights applied after.
  - One fp16 node table y = dinv1*x per layer, split A/B halves, AllGathered
    per layer (A fires mid-block-loop to overlap; B at layer end).
  - Main+ctrl graphs share one gather stream per (block, half): mixed chunks
    are consumed by two sel matmuls from one gathered tile. Main sel 0/1
    built on DVE (batched is_equal vs dst-local columns); ctrl sels
    (value = dinv2[src]/dinv1[src]) are host-built and streamed fp16.
  - Gather calls pack 8 chunks (1024 idxs = SWDGE ring cap), round-robin
    over 4 SWDGE queues. ~36ns/descriptor (HBM 256B random) is the wall.
v2.3 deltas vs v2 (1578us baseline -> ~1525us):
  - mC PSUM->SBUF copy moved DVE -> Scalar engine.
  - diag (self-loop) slabs host-precomputed, one DMA (kills 98 DVE
    tensor_scalar @ ~2.7us fixed cost each).
  - next-layer shard rows written straight into an SBUF ping-pong buffer
    (xsh for diags) and flushed to DRAM once per half for the AllGather:
    removes 147 per-block shard-write DMAs + 8 strided xsh reloads.
  - gather issue lookahead tuned to [4, 1] blocks (piece 0 / piece 1).
Measured dead ends (do not revisit without new evidence):
  - ctrl sels via DVE tensor_scalar: ~2.7us fixed cost/instr -> 2.4ms.
  - two pre-scaled tables (all-0/1 sels): 2x AllGather bytes + extra
    shard writes -> 2.0ms; CC traffic contends with gathers when moved
    into the compute window (staggered 3-piece tables -> 2.0ms).
  - piece split at 32 blocks: idx hits 32767 (int16 edge) -> flaky
    wrong results + slower. Keep piece row counts strictly < 4096/core.
  - single_packet=False on gathers: 1.89ms (worse).
  - SBUF-source transpose dma_gather: device INTERNAL crash on this
    runtime (microbench_gather.py), even with idx < 25600.
  - HW floor: 256B-random-HBM gather descriptors process at ~36ns/desc
    per SDMA engine (~90GB/s/core measured standalone); 549k descriptors
    over 4 layers ~= 1.24ms of the ~1.53ms span. Any big further win
    must cut descriptor count or leave this architecture.
"""

import math
import os

import numpy as np

F = 128
DEPTH = 4
P = 128
# table pieces (block ranges): staggered AllGathers; piece row counts must
# satisfy 8*rows <= 32768 (int16 gather idx)
PIECES = [(0, 25), (25, 49)]
NH = len(PIECES)
CALL_CAP = 8           # chunks per dma_gather call (1024 idxs = ring cap)

LAST_INFO = {}


class _Cfg:
    def __init__(self, n_nodes, n_cores=8):
        assert n_nodes % n_cores == 0
        self.N = n_nodes
        self.NCORES = n_cores
        self.SH = n_nodes // n_cores
        self.NBLK = math.ceil(self.SH / P)
        self.SHPAD = self.NBLK * P
        self.NQ = int(os.environ.get("GCN_NQUEUES", "4"))


def _prep_graph(edge_index, cfg):
    src = np.asarray(edge_index[0]).astype(np.int64)
    dst = np.asarray(edge_index[1]).astype(np.int64)
    deg = (np.bincount(dst, minlength=cfg.N) + 1).astype(np.float64)
    dinv = (1.0 / np.sqrt(deg)).astype(np.float32)

    core = dst // cfg.SH
    loc = dst - core * cfg.SH
    blk = loc // P
    dl = (loc % P).astype(np.int32)
    score = src // cfg.SH
    sloc = src - score * cfg.SH              # local row in source shard
    sblk = sloc // P
    half = np.zeros(len(src), np.int64)
    idxv = np.zeros(len(src), np.int64)
    for pi, (lo, hi) in enumerate(PIECES):
        m = (sblk >= lo) & (sblk < hi)
        half[m] = pi
        idxv[m] = score[m] * (hi - lo) * P + sloc[m] - lo * P
    idxv = idxv.astype(np.int32)
    assert idxv.max() < 32768
    return core, blk, half, idxv, dl, dinv, src


def _build_streams(x, edge_index, control_edge_index, cfg):
    """Host prep producing unified SPMD streams (cross-block call packing)."""
    graphs = [_prep_graph(edge_index, cfg),
              _prep_graph(control_edge_index, cfg)]
    dinvs = [graphs[0][5], graphs[1][5]]
    ratio = dinvs[1] / dinvs[0]   # dinv2/dinv1 per node

    buckets = {}
    for g in range(2):
        core, blk, half, idxv, dl, _, srcg = graphs[g]
        if g == 0:
            norm = np.ones(len(dl), np.float32)    # main sel value: 1.0
        else:
            norm = ratio[srcg]                     # ctrl: ratio[src]
        key = ((core * cfg.NBLK + blk) * NH + half)
        order = np.argsort(key, kind="stable")
        ks = key[order]
        bounds = np.searchsorted(
            ks, np.arange(cfg.NCORES * cfg.NBLK * NH + 1))
        for r in range(cfg.NCORES):
            for b in range(cfg.NBLK):
                for h in range(NH):
                    gi = (r * cfg.NBLK + b) * NH + h
                    sidx = order[bounds[gi]:bounds[gi + 1]]
                    buckets[(r, b, h, g)] = (idxv[sidx], dl[sidx], norm[sidx])

    # unified chunk structure per (b, h): nch = max over cores; per-chunk
    # graph-set = union over cores
    chmeta = {}            # (b, h) -> list of gsets
    for b in range(cfg.NBLK):
        for h in range(NH):
            nchs, bnds = [], []
            for r in range(cfg.NCORES):
                ne0 = len(buckets[(r, b, h, 0)][0])
                ne1 = len(buckets[(r, b, h, 1)][0])
                nchs.append(max(1, math.ceil((ne0 + ne1) / P)))
                bnds.append((ne0, ne0 + ne1))
            nch = max(nchs)
            gsets = []
            for c in range(nch):
                s0, s1 = c * P, (c + 1) * P
                gs = set()
                for r in range(cfg.NCORES):
                    ne0, ne = bnds[r]
                    if min(s1, ne0) > s0:
                        gs.add(0)
                    if min(s1, ne) > max(s0, ne0):
                        gs.add(1)
                if not gs:
                    gs = {0}
                gsets.append(sorted(gs))
            chmeta[(b, h)] = gsets

    # per-half chunk streams (block-ordered) -> calls of <= CALL_CAP chunks
    calls = [[] for _ in range(NH)]  # per h: (k, start_block, [(b, gset)])
    blockrefs = [[] for _ in range(cfg.NBLK)]   # per b: (h, ci, cc, gset)
    slot_of = {}           # (h, ci, cc) -> global slot index
    nslot = 0
    for h in range(NH):
        stream = [(b, gs) for b in range(cfg.NBLK)
                  for gs in chmeta[(b, h)]]
        c = 0
        while c < len(stream):
            k = min(CALL_CAP, len(stream) - c)
            ci = len(calls[h])
            chunks = stream[c:c + k]
            calls[h].append((k, chunks[0][0], chunks))
            for cc, (b, gs) in enumerate(chunks):
                blockrefs[b].append((h, ci, cc, gs))
                slot_of[(h, ci, cc)] = nslot
                nslot += 1
            c += k
    for b in range(cfg.NBLK):
        blockrefs[b].sort(key=lambda t: (t[0], t[1], t[2]))

    # ctrl sel stream grouped per block in blockref order; main sels are
    # built on-device (DVE is_equal vs dlm) so only dst-locals ship.
    nsel16 = sum(1 for b in range(cfg.NBLK) for (_h, _ci, _cc, gs)
                 in blockrefs[b] if 1 in gs)

    idx16s, dlms, sel16s = [], [], []
    for r in range(cfg.NCORES):
        idx_slots = np.zeros(nslot * P, np.int16)
        dlm_slots = np.full((P, nslot), 255.0, np.float16)
        selsB = np.zeros((nsel16, P, P), np.float16)
        chunk_pos = {}     # (b, h) -> next chunk ordinal within the group
        for h in range(NH):
            for ci, (k, _sb, chunks) in enumerate(calls[h]):
                for cc, (b, _gs) in enumerate(chunks):
                    j = chunk_pos.get((b, h), 0)
                    chunk_pos[(b, h)] = j + 1
                    i0, d0, _n0 = buckets[(r, b, h, 0)]
                    i1, _d1, _n1 = buckets[(r, b, h, 1)]
                    idxs = np.concatenate([i0, i1]).astype(np.int16)
                    lo, hi = j * P, min((j + 1) * P, len(idxs))
                    base = slot_of[(h, ci, cc)] * P
                    if hi > lo:
                        idx_slots[base:base + hi - lo] = idxs[lo:hi]
                    mhi = min(hi, len(i0))
                    if mhi > lo:
                        sl = slot_of[(h, ci, cc)]
                        dlm_slots[0:mhi - lo, sl] = d0[lo:mhi]
        # fill ctrl sels in blockref order
        si16 = 0
        for b in range(cfg.NBLK):
            seen = {}
            for (h, ci, cc, gs) in blockrefs[b]:
                j = seen.get(h, 0)
                seen[h] = j + 1
                i0, d0, n0 = buckets[(r, b, h, 0)]
                i1, d1, n1 = buckets[(r, b, h, 1)]
                dls = np.concatenate([d0, d1])
                nrms = np.concatenate([n0, n1])
                ne0, ne = len(i0), len(i0) + len(i1)
                s0, s1 = j * P, (j + 1) * P
                if 1 in gs:
                    lo = max(s0, ne0)
                    hi = min(s1, ne)
                    if hi > lo:
                        rows = np.arange(lo, hi) - s0
                        selsB[si16][rows, dls[lo:hi]] = nrms[lo:hi]
                    si16 += 1
        assert si16 == nsel16
        wrapped = idx_slots.reshape(-1, 16).T.copy()
        idx16s.append(np.tile(wrapped, (8, 1)))
        dlms.append(dlm_slots)
        sel16s.append(np.ascontiguousarray(
            selsB.transpose(1, 0, 2).reshape(P, nsel16 * P)))

    # layer-0 tables (one per piece, core-concatenated) + resident shard
    prow = [(hi - lo) * P for lo, hi in PIECES]
    xs = (np.asarray(x, np.float32) * dinvs[0][:, None]).astype(np.float32)
    xr = xs.reshape(cfg.NCORES, cfg.SH, F)
    xpads = [np.zeros((cfg.NCORES * prow[pi], F), np.float16)
             for pi in range(NH)]
    xsh0s = []
    for r in range(cfg.NCORES):
        full = np.zeros((cfg.SHPAD, F), np.float16)
        full[:cfg.SH] = xr[r].astype(np.float16)
        for pi, (lo, hi) in enumerate(PIECES):
            xpads[pi][r * prow[pi]:(r + 1) * prow[pi]] = \
                full[lo * P:hi * P]
        xsh0s.append(full)

    # host-precomputed diag slabs [P, NBLK*2, P]: diag per (block, graph):
    #   g=0: 1.0 (y row; end-scaled dinv1[d]); g=1: ratio[d]
    diags, dcols = [], []
    eye = np.eye(P, dtype=np.float32)
    for r in range(cfg.NCORES):
        dg = np.zeros((P, cfg.NBLK * 2, P), np.float16)
        dc_ = np.zeros((P, cfg.NBLK * 2), np.float32)
        for b in range(cfg.NBLK):
            i0 = r * cfg.SH + b * P
            n = min(P, cfg.SH - b * P)
            val1 = np.zeros(P, np.float32)
            val2 = np.zeros(P, np.float32)
            val1[:n] = 1.0
            val2[:n] = ratio[i0:i0 + n]
            dg[:, b * 2 + 0, :] = (eye * val1[:, None]).astype(np.float16)
            dg[:, b * 2 + 1, :] = (eye * val2[:, None]).astype(np.float16)
            dc_[:n, b * 2 + 0] = dinvs[0][i0:i0 + n]
            dc_[:n, b * 2 + 1] = dinvs[1][i0:i0 + n]
        diags.append(dg.reshape(P, cfg.NBLK * 2 * P))
        dcols.append(dc_)

    meta = dict(calls=calls, blockrefs=blockrefs, slot_of=slot_of,
                nslot=nslot, nsel16=nsel16)
    return (meta, idx16s, dlms, sel16s, xpads, xsh0s,
            diags, dcols)


def _build_program(cfg, meta, depth):
    import concourse.bacc as bacc
    import concourse.mybir as mybir
    import concourse.tile as tile

    dtH = mybir.dt.float16
    dt32 = mybir.dt.float32
    AT = mybir.AluOpType

    calls = meta["calls"]
    blockrefs = meta["blockrefs"]
    slot_of = meta["slot_of"]
    nslot = meta["nslot"]
    nsel16 = meta["nsel16"]

    nc = bacc.Bacc(
        "TRN2", debug=False, num_devices=cfg.NCORES,
        num_swdge_queues=cfg.NQ,
        dynamic_dma_scratch_size=16384,
    )

    prow = [(hi - lo) * P for lo, hi in PIECES]
    xpad_t = [nc.dram_tensor(f"xpad{pi}", [cfg.NCORES * prow[pi], F], dtH,
                             kind="ExternalInput")
              for pi in range(NH)]
    idx_t = nc.dram_tensor("idx16", [P, nslot * P // 16], mybir.dt.int16,
                           kind="ExternalInput")
    dlm_t = nc.dram_tensor("dlm", [P, nslot], dtH, kind="ExternalInput")
    sel16_t = nc.dram_tensor("sel16", [P, nsel16 * P], dtH,
                             kind="ExternalInput")
    diag_t = nc.dram_tensor("diag", [P, cfg.NBLK * 2 * P], dtH,
                            kind="ExternalInput")
    iotar_t = nc.dram_tensor("iotar", [P, P], dtH, kind="ExternalInput")
    w1_t = nc.dram_tensor("w1", [F, depth * F], dtH, kind="ExternalInput")
    w2_t = nc.dram_tensor("w2", [F, depth * F], dtH, kind="ExternalInput")
    xsh0_t = nc.dram_tensor("xsh0", [cfg.SHPAD, F], dtH, kind="ExternalInput")
    dcol_t = nc.dram_tensor("dcol", [P, cfg.NBLK * 2], dt32,
                            kind="ExternalInput")
    out_t = nc.dram_tensor("out", [cfg.SHPAD, F], dt32, kind="ExternalOutput")

    # per-block ctrl sel slab sizes
    sel16cnt, sel16off = [], []
    o16 = 0
    for b in range(cfg.NBLK):
        n16 = sum(1 for (_h, _ci, _cc, gs) in blockrefs[b] if 1 in gs)
        sel16off.append(o16)
        sel16cnt.append(n16)
        o16 += n16
    NS16MAX = max(sel16cnt)

    with tile.TileContext(nc) as tc:
        with (
            tc.tile_pool(name="const", bufs=1) as cpool,
            tc.tile_pool(name="gather", bufs=20) as gpool,
            tc.tile_pool(name="selb", bufs=12) as s8pool,
            tc.tile_pool(name="sel", bufs=4) as spool,
            tc.tile_pool(name="msb", bufs=4) as mpool,
            tc.tile_pool(name="xn", bufs=3) as xpool,
            tc.tile_pool(name="xsh", bufs=2) as xshpool,
            tc.tile_pool(name="pm", bufs=6, space="PSUM") as pmpool,
            tc.tile_pool(name="po", bufs=2, space="PSUM") as popool,
            tc.tile_pool(name="shard", bufs=6, space="DRAM") as shpool,
            tc.tile_pool(name="table", bufs=6, space="DRAM") as tbpool,
        ):
            idx_sb = cpool.tile([P, nslot * P // 16], mybir.dt.int16)
            nc.sync.dma_start(out=idx_sb[:], in_=idx_t[:])
            iotar_sb = cpool.tile([P, P], dtH)
            nc.sync.dma_start(out=iotar_sb[:], in_=iotar_t[:])
            iocall_sb = cpool.tile([P, CALL_CAP, P], dtH)
            for _c in range(CALL_CAP):
                nc.vector.tensor_copy(out=iocall_sb[:, _c, :],
                                      in_=iotar_sb[:])
            dlm_sb = cpool.tile([P, nslot], dtH)
            nc.sync.dma_start(out=dlm_sb[:], in_=dlm_t[:])
            w1_sb = cpool.tile([F, depth * F], dtH)
            w2_sb = cpool.tile([F, depth * F], dtH)
            nc.sync.dma_start(out=w1_sb[:], in_=w1_t[:])
            nc.sync.dma_start(out=w2_sb[:], in_=w2_t[:])
            dcol_sb = cpool.tile([P, cfg.NBLK * 2], dt32)
            nc.sync.dma_start(out=dcol_sb[:], in_=dcol_t[:])
            diag_sb = cpool.tile([P, cfg.NBLK * 2, P], dtH)
            nc.sync.dma_start(
                out=diag_sb[:],
                in_=diag_t[:].rearrange("p (b q) -> p b q", q=P))

            qrr = [0]
            prev_tab = [None] * NH
            xsh_cur = None
            for l in range(depth):
                if l < depth - 1:
                    sh = [shpool.tile([prow[pi], F], dtH, tag=f"sh{pi}",
                                      name=f"sh{pi}")
                          for pi in range(NH)]
                    # next-layer resident shard, filled block by block
                    xsh_nxt = xshpool.tile([P, cfg.NBLK, F], dtH, tag="xsh",
                                           name="xsh_nxt")
                if l == 0:
                    xsh_cur = xshpool.tile([P, cfg.NBLK, F], dtH, tag="xsh",
                                           name="xsh0")
                    nc.sync.dma_start(
                        out=xsh_cur[:],
                        in_=xsh0_t[:].rearrange("(b p) f -> p b f", p=P))

                next_tab = [None] * NH
                nxt = [0] * NH   # per-piece next-call-to-issue pointer
                gts = {}         # (h, ci) -> gather tile
                sel8s_t = {}     # (h, ci) -> built main-sel tile
                LAS = [4, 1]     # per-piece issue lookahead
                for b in range(cfg.NBLK):
                    sel16sb = spool.tile([P, NS16MAX, P], dtH, tag="sel16",
                                         name="sel16sb")
                    nc.sync.dma_start(
                        out=sel16sb[:, 0:sel16cnt[b], :],
                        in_=sel16_t[:, sel16off[b] * P:
                                    (sel16off[b] + sel16cnt[b]) * P
                                    ].rearrange("p (n q) -> p n q", q=P))

                    # issue gather calls (cross-block packing); early
                    # pieces run ahead (their tables land mid-prev-layer)
                    for h in range(NH):
                        lim = b + LAS[h]
                        while (nxt[h] < len(calls[h])
                               and calls[h][nxt[h]][1] <= lim):
                            ci = nxt[h]
                            k, _sb, _chunks = calls[h][ci]
                            gt = gpool.tile([P, CALL_CAP, F], dtH, tag="gt",
                                            name="gt")
                            L = k * P
                            s0 = slot_of[(h, ci, 0)]
                            if l == 0:
                                src_ap = xpad_t[h][:]
                            else:
                                src_ap = prev_tab[h][:]
                            nc.gpsimd.dma_gather(
                                gt[:, 0:k, :], src_ap,
                                idx_sb[:, s0 * 8:s0 * 8 + L // 16],
                                L, L, F,
                                queue_num=qrr[0] % cfg.NQ,
                            )
                            qrr[0] += 1
                            gts[(h, ci)] = gt
                            nxt[h] += 1

                    # build main 0/1 sels for calls this block consumes (DVE)
                    for (h, ci, _cc, _gs) in blockrefs[b]:
                        if (h, ci) in sel8s_t:
                            continue
                        k = calls[h][ci][0]
                        s0 = slot_of[(h, ci, 0)]
                        s8 = s8pool.tile([P, CALL_CAP, P], dtH,
                                         tag="sel8c", name="s8")
                        nc.vector.tensor_tensor(
                            out=s8[:, 0:k, :],
                            in0=iocall_sb[:, 0:k, :],
                            in1=dlm_sb[:, s0:s0 + k].to_broadcast(
                                [P, k, P]),
                            op=AT.is_equal,
                        )
                        sel8s_t[(h, ci)] = s8

                    # matmul accumulation: main split across 2 psums (A/B
                    # alternating), ctrl in psum C
                    pmA = pmpool.tile([P, P], dt32, tag="pm", name="pmA")
                    pmB = pmpool.tile([P, P], dt32, tag="pm", name="pmB")
                    pmC = pmpool.tile([P, P], dt32, tag="pm", name="pmC")
                    nA, nB, nC = 1, 0, 1
                    alt = 0
                    for (_h, _ci, _cc, gs) in blockrefs[b]:
                        for g in gs:
                            if g == 0:
                                if alt == 0:
                                    nA += 1
                                else:
                                    nB += 1
                                alt ^= 1
                            else:
                                nC += 1
                    use_B = nB > 0
                    dA, dB, dC = 0, 0, 0
                    nc.tensor.matmul(
                        out=pmA[:], lhsT=xsh_cur[:, b, :],
                        rhs=diag_sb[:, b * 2, :],
                        start=True, stop=(nA == 1),
                    )
                    dA = 1
                    nc.tensor.matmul(
                        out=pmC[:], lhsT=xsh_cur[:, b, :],
                        rhs=diag_sb[:, b * 2 + 1, :],
                        start=True, stop=(nC == 1),
                    )
                    dC = 1
                    sli16 = 0
                    alt = 0
                    for (h, ci, cc, gs) in blockrefs[b]:
                        gt = gts[(h, ci)]
                        for g in gs:
                            if g == 0:
                                rhs = sel8s_t[(h, ci)][:, cc, :]
                                if alt == 0:
                                    tgt = pmA
                                    dA += 1
                                    st = (dA == nA)
                                    first = False
                                else:
                                    tgt = pmB
                                    first = (dB == 0)
                                    dB += 1
                                    st = (dB == nB)
                                alt ^= 1
                            else:
                                rhs = sel16sb[:, sli16, :]
                                sli16 += 1
                                tgt = pmC
                                dC += 1
                                st = (dC == nC)
                                first = False
                            nc.tensor.matmul(
                                out=tgt[:],
                                lhsT=gt[:, cc, :],
                                rhs=rhs,
                                start=first,
                                stop=st,
                            )
                    mA = mpool.tile([P, P], dtH, tag="m", name="mA")
                    mC = mpool.tile([P, P], dtH, tag="m", name="mC")
                    nc.scalar.activation(
                        out=mA[:], in_=pmA[:],
                        func=mybir.ActivationFunctionType.Copy)
                    if use_B:
                        mB = mpool.tile([P, P], dtH, tag="m", name="mB")
                        nc.scalar.activation(
                            out=mB[:], in_=pmB[:],
                            func=mybir.ActivationFunctionType.Copy)
                    nc.scalar.activation(
                        out=mC[:], in_=pmC[:],
                        func=mybir.ActivationFunctionType.Copy)
                    poA = popool.tile([P, P], dt32, tag="po", name="poA")
                    poB = popool.tile([P, P], dt32, tag="po", name="poB")
                    nc.tensor.matmul(out=poA[:], lhsT=mA[:],
                                     rhs=w1_sb[:, l * F:(l + 1) * F],
                                     start=True, stop=not use_B)
                    if use_B:
                        nc.tensor.matmul(out=poA[:], lhsT=mB[:],
                                         rhs=w1_sb[:, l * F:(l + 1) * F],
                                         start=False, stop=True)
                    nc.tensor.matmul(out=poB[:], lhsT=mC[:],
                                     rhs=w2_sb[:, l * F:(l + 1) * F],
                                     start=True, stop=True)
                    # pB = poB * dinv2[dst] (PSUM->SBUF); pt = poA*dinv1 + pB
                    pB = xpool.tile([P, P], dt32, tag="pB", name="pB")
                    nc.scalar.activation(
                        out=pB[:], in_=poB[:],
                        func=mybir.ActivationFunctionType.Copy,
                        scale=dcol_sb[:, b * 2 + 1:b * 2 + 2])
                    pt = xpool.tile([P, P], dt32, tag="pt", name="pt")
                    nc.vector.scalar_tensor_tensor(
                        out=pt[:], in0=poA[:],
                        scalar=dcol_sb[:, b * 2:b * 2 + 1],
                        in1=pB[:], op0=AT.mult, op1=AT.add)
                    if l < depth - 1:
                        # next-layer shard row y1 = relu(dinv1[d] * pt),
                        # written straight into the SBUF ping-pong shard
                        nc.scalar.activation(
                            out=xsh_nxt[:, b, :], in_=pt[:],
                            func=mybir.ActivationFunctionType.Relu,
                            scale=dcol_sb[:, b * 2:b * 2 + 1])
                        for pi, (plo, phi) in enumerate(PIECES):
                            if b != phi - 1:
                                continue
                            # piece complete: flush once + AllGather,
                            # overlapped with remaining blocks' compute
                            nc.sync.dma_start(
                                out=sh[pi][:].rearrange(
                                    "(b p) f -> p b f", p=P),
                                in_=xsh_nxt[:, plo:phi, :])
                            tab = tbpool.tile(
                                [cfg.NCORES * prow[pi], F], dtH,
                                tag=f"tab{pi}", name=f"tab{pi}",
                                addr_space="Shared")
                            nc.gpsimd.collective_compute(
                                "AllGather",
                                mybir.AluOpType.bypass,
                                replica_groups=[list(range(cfg.NCORES))],
                                ins=[sh[pi].opt()],
                                outs=[tab.opt()],
                            )
                            next_tab[pi] = tab
                    else:
                        nc.sync.dma_start(
                            out=out_t[b * P:(b + 1) * P, :], in_=pt[:])

                if l < depth - 1:
                    prev_tab = next_tab
                    xsh_cur = xsh_nxt

    nc.compile()
    return nc


def _run(x, edge_index, control_edge_index, conv_w, conv_b, ctrl_w, ctrl_b,
         cfg, trace=False):
    from concourse.bass_utils import run_bass_kernel_spmd

    depth = int(np.asarray(conv_w).shape[0])
    (meta, idx16s, dlms, sel16s, xpads,
     xsh0s, diags, dcols) = _build_streams(
        x, edge_index, control_edge_index, cfg)
    bsum = (np.asarray(conv_b, np.float32)
            + np.asarray(ctrl_b, np.float32))
    assert not np.any(bsum), "bias path not supported"
    nc = _build_program(cfg, meta, depth)

    w1 = (np.asarray(conv_w, np.float32).transpose(1, 0, 2)
          .reshape(F, depth * F).astype(np.float16))
    w2 = (np.asarray(ctrl_w, np.float32).transpose(1, 0, 2)
          .reshape(F, depth * F).astype(np.float16))
    iotar = np.tile(np.arange(P, dtype=np.float16), (P, 1))

    in_maps = []
    for r in range(cfg.NCORES):
        m = {"idx16": idx16s[r],
             "dlm": dlms[r], "sel16": sel16s[r], "diag": diags[r],
             "w1": w1, "w2": w2, "iotar": iotar,
             "xsh0": xsh0s[r], "dcol": dcols[r]}
        for pi in range(NH):
            m[f"xpad{pi}"] = xpads[pi]
        in_maps.append(m)

    try:
        res = run_bass_kernel_spmd(nc, in_maps, list(range(cfg.NCORES)),
                                   trace=trace)
    except Exception:
        if not trace:
            raise
        res = run_bass_kernel_spmd(nc, in_maps, list(range(cfg.NCORES)),
                                   trace=False)
    LAST_INFO.clear()
    LAST_INFO["exec_time_ns"] = res.exec_time_ns
    LAST_INFO["mean_exec_time_ns"] = res.mean_exec_time_ns
    LAST_INFO["profile_json"] = res.profile_json

    out = np.empty((cfg.N, F), np.float32)
    for r in range(cfg.NCORES):
        out[r * cfg.SH:(r + 1) * cfg.SH] = res.results[r]["out"][:cfg.SH]
    return out


def kernel(x, edge_index, control_edge_index, conv_w, conv_b, ctrl_w, ctrl_b):
    cfg = _Cfg(int(np.asarray(x).shape[0]))
    trace = os.environ.get("GCN_TRACE", "0") == "1"
    return _run(x, edge_index, control_edge_index, conv_w, conv_b,
                ctrl_w, ctrl_b, cfg, trace=trace)
